# revision 19
# baseline (speedup 1.0000x reference)
"""ChannelGuidedAttn Trainium2 kernel.

Reference computation (per batch b):
    q  = x_pre[b]      reshaped (C, N),  C=512, N=H*W=4096
    kv = x_training[b] reshaped (C, N)
    energy[c,d] = <q[c,:], kv[d,:]>                      (C x C)
    att = softmax(max_d(energy) - energy, axis=-1)       == softmax(-energy)
        = exp(min_d(energy) - energy) / sum
    out = att @ kv  -> (C, H, W);  final softmax over W

Sharding: data-parallel over batch B=16 across 8 cores (2 batches/core).

Per-core kernel (Tile framework):
  - gemm1 needs both operands with n on partitions -> fp16 cast + xbar
    DMA-transpose (2-byte dtype) of q and kv.
  - precision: fp16 hi/lo split for gemm1 (energy = qh.kh + ql.kh + qh.kl
    accumulated in fp32 PSUM) -> ~8.5e-4 final absmax rel err.
  - gemm2 uses att^T (small DMA-transpose) against kv in natural layout.
  - final softmax over W=64 segments: exp on ACT into SBUF (no max-subtract
    needed; |out| < ~6), strided segment reduce + in-place broadcast multiply
    on DVE at fp32-2x SBUF rate.
  - engine balance: casts on GPSIMD (kv) and ACT (q), residual subs on DVE,
    exp on ACT, reduces/normalize on DVE.
  - cross-batch overlap: kv natural tiles are per-d-tile (pool bufs=5) and
    next batch's loads/casts overlap current batch's gemms; the unified
    khT/klT transposed tensors are single-buffered (their rebuild waits only
    for the last gemm1 of the previous batch).
"""

import sys

import numpy as np

for _p in ("/opt/trn_rl_repo", "/root/.axon_site/_ro/trn_rl_repo"):
    if _p not in sys.path:
        sys.path.append(_p)

B = 16
N_CORES = 8
B_PER_CORE = B // N_CORES
C = 512
H = 64
W = 64
N = H * W
CT = C // 128  # 4 c-tiles / d-tiles
NJ = N // 128  # 32 n-chunks of 128
NO = N // 512  # 8 output chunks of 512

G1_MODE = "full"  # "full" (3-matmul split) | "qsplit" (2-matmul) | "plain"


def build_program(g1_mode=None):
    from contextlib import ExitStack

    import concourse.mybir as mybir
    import concourse.tile as tile
    from concourse import bacc

    if g1_mode is None:
        g1_mode = G1_MODE
    assert g1_mode in ("full", "qsplit", "plain")
    kv_split = g1_mode == "full"
    q_split = g1_mode in ("full", "qsplit")

    f32 = mybir.dt.float32
    f16 = mybir.dt.float16
    Alu = mybir.AluOpType
    Act = mybir.ActivationFunctionType
    Axis = mybir.AxisListType

    nc = bacc.Bacc()
    xt = nc.declare_dram_parameter("xt", [B_PER_CORE, C, N], f32, isOutput=False)
    xp = nc.declare_dram_parameter("xp", [B_PER_CORE, C, N], f32, isOutput=False)
    out = nc.declare_dram_parameter("out", [B_PER_CORE, C, N], f32, isOutput=True)

    with tile.TileContext(nc) as tc, ExitStack() as ctx:
        raw = ctx.enter_context(tc.tile_pool(name="raw", bufs=3))
        stage16 = ctx.enter_context(tc.tile_pool(name="stage16", bufs=4))
        knat = ctx.enter_context(tc.tile_pool(name="knat", bufs=7))
        ktp = ctx.enter_context(tc.tile_pool(name="ktp", bufs=1))
        qtpool = ctx.enter_context(tc.tile_pool(name="qtpool", bufs=2))
        att_pool = ctx.enter_context(tc.tile_pool(name="att", bufs=2))
        small = ctx.enter_context(tc.tile_pool(name="small", bufs=3))
        opool = ctx.enter_context(tc.tile_pool(name="opool", bufs=2))
        ps_e = ctx.enter_context(tc.tile_pool(name="ps_e", bufs=2, space="PSUM"))
        ps_o = ctx.enter_context(tc.tile_pool(name="ps_o", bufs=6, space="PSUM"))

        for b in range(B_PER_CORE):
            # ---- kv prep: per-d-tile natural fp16, unified transposed ----
            kh_nat = [
                knat.tile([128, N], f16, tag="kh_nat", name=f"kh_nat_{b}_{i}")
                for i in range(CT)
            ]
            khT = ktp.tile([128, CT, NJ, 128], f16, tag="khT")
            if kv_split:
                klT = ktp.tile([128, CT, NJ, 128], f16, tag="klT")
            for dt in range(CT):
                for h in range(2):
                    hs = slice(h * (N // 2), (h + 1) * (N // 2))
                    js = slice(h * (NJ // 2), (h + 1) * (NJ // 2))
                    kv_f32 = raw.tile([128, N // 2], f32, tag="raw")
                    nc.sync.dma_start(
                        out=kv_f32, in_=xt[b, dt * 128 : (dt + 1) * 128, hs]
                    )
                    nc.gpsimd.tensor_copy(out=kh_nat[dt][:, hs], in_=kv_f32)
                    nc.sync.dma_start_transpose(khT[:, dt, js], kh_nat[dt][:, hs])
                    if kv_split:
                        kl_st = stage16.tile([128, N // 2], f16, tag="stage16")
                        nc.vector.tensor_tensor(
                            out=kl_st, in0=kv_f32, in1=kh_nat[dt][:, hs], op=Alu.subtract
                        )
                        nc.sync.dma_start_transpose(klT[:, dt, js], kl_st)

            for ct in range(CT):
                # ---- q prep for this c-tile ----
                qhT = qtpool.tile([128, NJ, 128], f16, tag="qhT")
                if q_split:
                    qlT = qtpool.tile([128, NJ, 128], f16, tag="qlT")
                for h in range(2):
                    hs = slice(h * (N // 2), (h + 1) * (N // 2))
                    js = slice(h * (NJ // 2), (h + 1) * (NJ // 2))
                    q_f32 = raw.tile([128, N // 2], f32, tag="raw")
                    nc.sync.dma_start(
                        out=q_f32, in_=xp[b, ct * 128 : (ct + 1) * 128, hs]
                    )
                    qh_st = stage16.tile([128, N // 2], f16, tag="stage16")
                    nc.scalar.copy(out=qh_st, in_=q_f32)
                    nc.sync.dma_start_transpose(qhT[:, js], qh_st)
                    if q_split:
                        ql_st = stage16.tile([128, N // 2], f16, tag="stage16")
                        nc.vector.tensor_tensor(
                            out=ql_st, in0=q_f32, in1=qh_st, op=Alu.subtract
                        )
                        nc.sync.dma_start_transpose(qlT[:, js], ql_st)

                # ---- gemm1: energy[c_tile, :] accumulated over n-chunks ----
                e_ps = ps_e.tile([128, C], f32, tag="ps_e")
                for j in range(NJ):
                    last = j == NJ - 1
                    nc.tensor.matmul(
                        e_ps,
                        qhT[:, j, :],
                        khT[:, :, j, :],
                        start=(j == 0),
                        stop=(last and not q_split),
                    )
                    if q_split:
                        nc.tensor.matmul(
                            e_ps,
                            qlT[:, j, :],
                            khT[:, :, j, :],
                            start=False,
                            stop=(last and not kv_split),
                        )
                    if kv_split:
                        nc.tensor.matmul(
                            e_ps, qhT[:, j, :], klT[:, :, j, :], start=False, stop=last
                        )

                # ---- softmax over d (free axis): att = exp(min - E)/sum ----
                min_t = small.tile([128, 1], f32, tag="min")
                nc.vector.tensor_reduce(min_t, e_ps, axis=Axis.X, op=Alu.min)
                att16 = att_pool.tile([128, C], f16, tag="att16")
                den = small.tile([128, 1], f32, tag="den")
                nc.scalar.activation(
                    out=att16,
                    in_=e_ps,
                    func=Act.Exp,
                    bias=min_t,
                    scale=-1.0,
                    accum_out=den,
                )
                rden = small.tile([128, 1], f32, tag="rden")
                nc.vector.reciprocal(rden, den)
                nc.vector.tensor_scalar_mul(att16, att16, rden)
                attT = att_pool.tile([128, CT, 128], f16, tag="attT")
                nc.sync.dma_start_transpose(attT, att16)

                # ---- gemm2 + final softmax over W segments ----
                for nj in range(NO):
                    o_ps = ps_o.tile([128, 512], f32, tag="ps_o")
                    for dt in range(CT):
                        nc.tensor.matmul(
                            o_ps,
                            attT[:, dt, :],
                            kh_nat[dt][:, nj * 512 : (nj + 1) * 512],
                            start=(dt == 0),
                            stop=(dt == CT - 1),
                        )
                    o_sb = opool.tile([128, 512 // W, W], f32, tag="osb")
                    nc.scalar.activation(
                        out=o_sb,
                        in_=o_ps.rearrange("p (s w) -> p s w", w=W),
                        func=Act.Exp,
                    )
                    ssum = small.tile([128, 512 // W], f32, tag="ssum")
                    nc.vector.tensor_reduce(ssum, o_sb, axis=Axis.X, op=Alu.add)
                    rsum = small.tile([128, 512 // W], f32, tag="rsum")
                    nc.vector.reciprocal(rsum, ssum)
                    nc.vector.tensor_tensor(
                        out=o_sb,
                        in0=o_sb,
                        in1=rsum[:, :, None].to_broadcast(o_sb.shape),
                        op=Alu.mult,
                    )
                    nc.sync.dma_start(
                        out=out[
                            b, ct * 128 : (ct + 1) * 128, nj * 512 : (nj + 1) * 512
                        ],
                        in_=o_sb,
                    )

    nc.finalize()
    return nc


def kernel(x_training: np.ndarray, x_pre: np.ndarray) -> np.ndarray:
    from concourse.bass_utils import run_bass_kernel_spmd

    nc = build_program()

    xt = np.ascontiguousarray(
        np.asarray(x_training, dtype=np.float32).reshape(B, C, N)
    )
    xp = np.ascontiguousarray(np.asarray(x_pre, dtype=np.float32).reshape(B, C, N))

    in_maps = []
    for i in range(N_CORES):
        sl = slice(i * B_PER_CORE, (i + 1) * B_PER_CORE)
        in_maps.append({"xt": xt[sl], "xp": xp[sl]})

    res = run_bass_kernel_spmd(nc, in_maps, list(range(N_CORES)))
    outs = [np.asarray(r["out"]) for r in res.results]
    return np.concatenate(outs, axis=0).reshape(B, C, H, W).astype(np.float32)


# revision 24
# speedup vs baseline: 17762.3337x; 17762.3337x over previous
"""ChannelGuidedAttn Trainium2 kernel.

Reference computation (per batch b):
    q  = x_pre[b]      reshaped (C, N),  C=512, N=H*W=4096
    kv = x_training[b] reshaped (C, N)
    energy[c,d] = <q[c,:], kv[d,:]>                      (C x C)
    att = softmax(max_d(energy) - energy, axis=-1)       == softmax(-energy)
        = exp(min_d(energy) - energy) / sum
    out = att @ kv  -> (C, H, W);  final softmax over W

Sharding: data-parallel over batch B=16 across 8 cores (2 batches/core).

Per-core kernel (Tile framework):
  - gemm1 needs both operands with n on partitions -> fp16 cast + xbar
    DMA-transpose (2-byte dtype) of q and kv.
  - precision: fp16 hi/lo split for gemm1 (energy = qh.kh + ql.kh + qh.kl
    accumulated in fp32 PSUM) -> ~8.5e-4 final absmax rel err.
  - gemm2 uses att^T (small DMA-transpose) against kv in natural layout.
  - final softmax over W=64 segments: exp on ACT into SBUF (no max-subtract
    needed; |out| < ~6), strided segment reduce + in-place broadcast multiply
    on DVE at fp32-2x SBUF rate.
  - engine balance: casts on GPSIMD (kv) and ACT (q), residual subs on DVE,
    exp on ACT, reduces/normalize on DVE.
  - cross-batch overlap: kv natural tiles are per-d-tile (pool bufs=7) and
    next batch's loads/casts overlap current batch's gemms; the unified
    khT/klT transposed tensors are single-buffered (their rebuild waits only
    for the last gemm1 of the previous batch).
"""

import sys

import numpy as np

for _p in ("/opt/trn_rl_repo", "/root/.axon_site/_ro/trn_rl_repo"):
    if _p not in sys.path:
        sys.path.append(_p)

B = 16
N_CORES = 8
B_PER_CORE = B // N_CORES
C = 512
H = 64
W = 64
N = H * W
CT = C // 128  # 4 c-tiles / d-tiles
NJ = N // 128  # 32 n-chunks of 128
NO = N // 512  # 8 output chunks of 512

G1_MODE = "full"  # "full" (3-matmul split) | "qsplit" (2-matmul) | "plain"


def build_program(g1_mode=None):
    from contextlib import ExitStack

    import concourse.mybir as mybir
    import concourse.tile as tile
    from concourse import bacc

    if g1_mode is None:
        g1_mode = G1_MODE
    assert g1_mode in ("full", "qsplit", "plain")
    kv_split = g1_mode == "full"
    q_split = g1_mode in ("full", "qsplit")

    f32 = mybir.dt.float32
    f16 = mybir.dt.float16
    Alu = mybir.AluOpType
    Act = mybir.ActivationFunctionType
    Axis = mybir.AxisListType

    nc = bacc.Bacc()
    xt = nc.declare_dram_parameter("xt", [B_PER_CORE, C, N], f32, isOutput=False)
    xp = nc.declare_dram_parameter("xp", [B_PER_CORE, C, N], f32, isOutput=False)
    out = nc.declare_dram_parameter("out", [B_PER_CORE, C, N], f32, isOutput=True)

    with tile.TileContext(nc) as tc, ExitStack() as ctx:
        raw = ctx.enter_context(tc.tile_pool(name="raw", bufs=3))
        stage16 = ctx.enter_context(tc.tile_pool(name="stage16", bufs=4))
        knat = ctx.enter_context(tc.tile_pool(name="knat", bufs=7))
        ktp = ctx.enter_context(tc.tile_pool(name="ktp", bufs=1))
        qtpool = ctx.enter_context(tc.tile_pool(name="qtpool", bufs=2))
        att_pool = ctx.enter_context(tc.tile_pool(name="att", bufs=2))
        small = ctx.enter_context(tc.tile_pool(name="small", bufs=3))
        opool = ctx.enter_context(tc.tile_pool(name="opool", bufs=2))
        ps_e = ctx.enter_context(tc.tile_pool(name="ps_e", bufs=2, space="PSUM"))
        ps_o = ctx.enter_context(tc.tile_pool(name="ps_o", bufs=6, space="PSUM"))

        for b in range(B_PER_CORE):
            # ---- kv prep: per-d-tile natural fp16, unified transposed ----
            kh_nat = [
                knat.tile([128, N], f16, tag="kh_nat", name=f"kh_nat_{b}_{i}")
                for i in range(CT)
            ]
            khT = ktp.tile([128, CT, NJ, 128], f16, tag="khT")
            if kv_split:
                klT = ktp.tile([128, CT, NJ, 128], f16, tag="klT")
            for dt in range(CT):
                for h in range(2):
                    hs = slice(h * (N // 2), (h + 1) * (N // 2))
                    js = slice(h * (NJ // 2), (h + 1) * (NJ // 2))
                    kv_f32 = raw.tile([128, N // 2], f32, tag="raw")
                    nc.sync.dma_start(
                        out=kv_f32, in_=xt[b, dt * 128 : (dt + 1) * 128, hs]
                    )
                    nc.gpsimd.tensor_copy(out=kh_nat[dt][:, hs], in_=kv_f32)
                    nc.sync.dma_start_transpose(khT[:, dt, js], kh_nat[dt][:, hs])
                    if kv_split:
                        kl_st = stage16.tile([128, N // 2], f16, tag="stage16")
                        nc.vector.tensor_tensor(
                            out=kl_st, in0=kv_f32, in1=kh_nat[dt][:, hs], op=Alu.subtract
                        )
                        nc.sync.dma_start_transpose(klT[:, dt, js], kl_st)

            for ct in range(CT):
                # ---- q prep for this c-tile ----
                qhT = qtpool.tile([128, NJ, 128], f16, tag="qhT")
                if q_split:
                    qlT = qtpool.tile([128, NJ, 128], f16, tag="qlT")
                for h in range(2):
                    hs = slice(h * (N // 2), (h + 1) * (N // 2))
                    js = slice(h * (NJ // 2), (h + 1) * (NJ // 2))
                    q_f32 = raw.tile([128, N // 2], f32, tag="raw")
                    nc.sync.dma_start(
                        out=q_f32, in_=xp[b, ct * 128 : (ct + 1) * 128, hs]
                    )
                    qh_st = stage16.tile([128, N // 2], f16, tag="stage16")
                    nc.scalar.copy(out=qh_st, in_=q_f32)
                    nc.sync.dma_start_transpose(qhT[:, js], qh_st)
                    if q_split:
                        ql_st = stage16.tile([128, N // 2], f16, tag="stage16")
                        nc.vector.tensor_tensor(
                            out=ql_st, in0=q_f32, in1=qh_st, op=Alu.subtract
                        )
                        nc.sync.dma_start_transpose(qlT[:, js], ql_st)

                # ---- gemm1: energy[c_tile, :] accumulated over n-chunks ----
                e_ps = ps_e.tile([128, C], f32, tag="ps_e")
                for j in range(NJ):
                    last = j == NJ - 1
                    nc.tensor.matmul(
                        e_ps,
                        qhT[:, j, :],
                        khT[:, :, j, :],
                        start=(j == 0),
                        stop=(last and not q_split),
                    )
                    if q_split:
                        nc.tensor.matmul(
                            e_ps,
                            qlT[:, j, :],
                            khT[:, :, j, :],
                            start=False,
                            stop=(last and not kv_split),
                        )
                    if kv_split:
                        nc.tensor.matmul(
                            e_ps, qhT[:, j, :], klT[:, :, j, :], start=False, stop=last
                        )

                # ---- softmax over d (free axis): att = exp(min - E)/sum ----
                min_t = small.tile([128, 1], f32, tag="min")
                nc.vector.tensor_reduce(min_t, e_ps, axis=Axis.X, op=Alu.min)
                att16 = att_pool.tile([128, C], f16, tag="att16")
                den = small.tile([128, 1], f32, tag="den")
                nc.scalar.activation(
                    out=att16,
                    in_=e_ps,
                    func=Act.Exp,
                    bias=min_t,
                    scale=-1.0,
                    accum_out=den,
                )
                rden = small.tile([128, 1], f32, tag="rden")
                nc.vector.reciprocal(rden, den)
                nc.vector.tensor_scalar_mul(att16, att16, rden)
                attT = att_pool.tile([128, CT, 128], f16, tag="attT")
                nc.sync.dma_start_transpose(attT, att16)

                # ---- gemm2 + final softmax over W segments ----
                for nj in range(NO):
                    o_ps = ps_o.tile([128, 512], f32, tag="ps_o")
                    for dt in range(CT):
                        nc.tensor.matmul(
                            o_ps,
                            attT[:, dt, :],
                            kh_nat[dt][:, nj * 512 : (nj + 1) * 512],
                            start=(dt == 0),
                            stop=(dt == CT - 1),
                        )
                    o_sb = opool.tile([128, 512 // W, W], f32, tag="osb")
                    nc.scalar.activation(
                        out=o_sb,
                        in_=o_ps.rearrange("p (s w) -> p s w", w=W),
                        func=Act.Exp,
                    )
                    ssum = small.tile([128, 512 // W], f32, tag="ssum")
                    nc.vector.tensor_reduce(ssum, o_sb, axis=Axis.X, op=Alu.add)
                    rsum = small.tile([128, 512 // W], f32, tag="rsum")
                    nc.vector.reciprocal(rsum, ssum)
                    nc.vector.tensor_tensor(
                        out=o_sb,
                        in0=o_sb,
                        in1=rsum[:, :, None].to_broadcast(o_sb.shape),
                        op=Alu.mult,
                    )
                    nc.sync.dma_start(
                        out=out[
                            b, ct * 128 : (ct + 1) * 128, nj * 512 : (nj + 1) * 512
                        ],
                        in_=o_sb,
                    )

    nc.finalize()
    return nc


def kernel(x_training: np.ndarray, x_pre: np.ndarray) -> np.ndarray:
    from concourse.bass_utils import run_bass_kernel_spmd

    nc = build_program()

    xt = np.ascontiguousarray(
        np.asarray(x_training, dtype=np.float32).reshape(B, C, N)
    )
    xp = np.ascontiguousarray(np.asarray(x_pre, dtype=np.float32).reshape(B, C, N))

    in_maps = []
    for i in range(N_CORES):
        sl = slice(i * B_PER_CORE, (i + 1) * B_PER_CORE)
        in_maps.append({"xt": xt[sl], "xp": xp[sl]})

    res = run_bass_kernel_spmd(nc, in_maps, list(range(N_CORES)))
    outs = [np.asarray(r["out"]) for r in res.results]
    return np.concatenate(outs, axis=0).reshape(B, C, H, W).astype(np.float32)


# revision 33
# speedup vs baseline: 19082.9873x; 1.0744x over previous
"""ChannelGuidedAttn Trainium2 kernel.

Reference computation (per batch b):
    q  = x_pre[b]      reshaped (C, N),  C=512, N=H*W=4096
    kv = x_training[b] reshaped (C, N)
    energy[c,d] = <q[c,:], kv[d,:]>                      (C x C)
    att = softmax(max_d(energy) - energy, axis=-1)       == softmax(-energy)
        = exp(min_d(energy) - energy) / sum
    out = att @ kv  -> (C, H, W);  final softmax over W

Sharding: data-parallel over batch B=16 across 8 cores (2 batches/core).

Per-core kernel (Tile framework):
  - gemm1 needs both operands with n on partitions -> fp16 cast + xbar
    DMA-transpose (2-byte dtype) of q and kv.
  - precision: fp16 hi/lo split for gemm1 (energy = qh.kh + ql.kh + qh.kl
    accumulated in fp32 PSUM) -> ~8.5e-4 final absmax rel err.
  - gemm2 uses att^T (small DMA-transpose) against kv in natural layout.
  - final softmax over W=64 segments: exp on ACT into SBUF (no max-subtract
    needed; |out| < ~6), strided segment reduce + in-place broadcast multiply
    on DVE at fp32-2x SBUF rate.
  - engine balance: casts on GPSIMD (kv) / ACT (q), residual subs on GPSIMD,
    exp on ACT, reduces/normalize on DVE.
  - the kv lo-residual transpose (klT) runs on the PE (is_transpose matmuls +
    PSUM->SBUF copies alternating DVE/ACT) instead of the DMA xbar: it is off
    gemm1's critical path (3rd accumulation pass) and shifts 4MB/batch off
    the DMA queues, which are the bottleneck.
  - cross-batch overlap: kv natural tiles are per-d-tile (pool bufs=7) and
    next batch's loads/casts overlap current batch's gemms; the unified
    khT/klT transposed tensors are single-buffered (their rebuild waits only
    for the last gemm1 of the previous batch).
"""

import sys

import numpy as np

for _p in ("/opt/trn_rl_repo", "/root/.axon_site/_ro/trn_rl_repo"):
    if _p not in sys.path:
        sys.path.append(_p)

B = 16
N_CORES = 8
B_PER_CORE = B // N_CORES
C = 512
H = 64
W = 64
N = H * W
CT = C // 128  # 4 c-tiles / d-tiles
NJ = N // 128  # 32 n-chunks of 128
NO = N // 512  # 8 output chunks of 512

G1_MODE = "full"  # "full" (3-matmul split) | "qsplit" (2-matmul) | "plain"


def build_program(g1_mode=None):
    from contextlib import ExitStack

    import concourse.mybir as mybir
    import concourse.tile as tile
    from concourse import bacc

    if g1_mode is None:
        g1_mode = G1_MODE
    assert g1_mode in ("full", "qsplit", "plain")
    kv_split = g1_mode == "full"
    q_split = g1_mode in ("full", "qsplit")

    f32 = mybir.dt.float32
    f16 = mybir.dt.float16
    Alu = mybir.AluOpType
    Act = mybir.ActivationFunctionType
    Axis = mybir.AxisListType

    nc = bacc.Bacc()
    xt = nc.declare_dram_parameter("xt", [B_PER_CORE, C, N], f32, isOutput=False)
    xp = nc.declare_dram_parameter("xp", [B_PER_CORE, C, N], f32, isOutput=False)
    out = nc.declare_dram_parameter("out", [B_PER_CORE, C, N], f32, isOutput=True)

    with tile.TileContext(nc) as tc, ExitStack() as ctx:
        raw = ctx.enter_context(tc.tile_pool(name="raw", bufs=3))
        stage16 = ctx.enter_context(tc.tile_pool(name="stage16", bufs=4))
        knat = ctx.enter_context(tc.tile_pool(name="knat", bufs=7))
        ktp = ctx.enter_context(tc.tile_pool(name="ktp", bufs=1))
        qtpool = ctx.enter_context(tc.tile_pool(name="qtpool", bufs=2))
        att_pool = ctx.enter_context(tc.tile_pool(name="att", bufs=2))
        small = ctx.enter_context(tc.tile_pool(name="small", bufs=3))
        opool = ctx.enter_context(tc.tile_pool(name="opool", bufs=2))
        ps_e = ctx.enter_context(tc.tile_pool(name="ps_e", bufs=2, space="PSUM"))
        ps_t = ctx.enter_context(tc.tile_pool(name="ps_t", bufs=2, space="PSUM"))
        ps_o = ctx.enter_context(tc.tile_pool(name="ps_o", bufs=4, space="PSUM"))

        from concourse import masks

        ident = att_pool.tile([128, 128], f16, tag="ident")
        masks.make_identity(nc, ident)

        for b in range(B_PER_CORE):
            # ---- kv prep: per-d-tile natural fp16, unified transposed ----
            kh_nat = [
                knat.tile([128, N], f16, tag="kh_nat", name=f"kh_nat_{b}_{i}")
                for i in range(CT)
            ]
            khT = ktp.tile([128, CT, NJ, 128], f16, tag="khT")
            if kv_split:
                klT = ktp.tile([128, CT, NJ, 128], f16, tag="klT")
            for dt in range(CT):
                for h in range(2):
                    hs = slice(h * (N // 2), (h + 1) * (N // 2))
                    js = slice(h * (NJ // 2), (h + 1) * (NJ // 2))
                    kv_f32 = raw.tile([128, N // 2], f32, tag="raw")
                    nc.sync.dma_start(
                        out=kv_f32, in_=xt[b, dt * 128 : (dt + 1) * 128, hs]
                    )
                    nc.gpsimd.tensor_copy(out=kh_nat[dt][:, hs], in_=kv_f32)
                    nc.sync.dma_start_transpose(khT[:, dt, js], kh_nat[dt][:, hs])
                    if kv_split:
                        kl_st = stage16.tile([128, N // 2], f16, tag="stage16")
                        nc.gpsimd.tensor_tensor(
                            out=kl_st, in0=kv_f32, in1=kh_nat[dt][:, hs], op=Alu.subtract
                        )
                        for i in range(NJ // 2):
                            pst = ps_t.tile([128, 128], f16, tag="pst")
                            nc.tensor.transpose(
                                pst, kl_st[:, i * 128 : (i + 1) * 128], ident
                            )
                            if i % 2 == 0:
                                nc.vector.tensor_copy(
                                    out=klT[:, dt, h * (NJ // 2) + i, :], in_=pst
                                )
                            else:
                                nc.scalar.copy(
                                    out=klT[:, dt, h * (NJ // 2) + i, :], in_=pst
                                )

            for ct in range(CT):
                # ---- q prep for this c-tile ----
                qhT = qtpool.tile([128, NJ, 128], f16, tag="qhT")
                if q_split:
                    qlT = qtpool.tile([128, NJ, 128], f16, tag="qlT")
                for h in range(2):
                    hs = slice(h * (N // 2), (h + 1) * (N // 2))
                    js = slice(h * (NJ // 2), (h + 1) * (NJ // 2))
                    q_f32 = raw.tile([128, N // 2], f32, tag="raw")
                    nc.sync.dma_start(
                        out=q_f32, in_=xp[b, ct * 128 : (ct + 1) * 128, hs]
                    )
                    qh_st = stage16.tile([128, N // 2], f16, tag="stage16")
                    nc.scalar.copy(out=qh_st, in_=q_f32)
                    nc.sync.dma_start_transpose(qhT[:, js], qh_st)
                    if q_split:
                        ql_st = stage16.tile([128, N // 2], f16, tag="stage16")
                        nc.gpsimd.tensor_tensor(
                            out=ql_st, in0=q_f32, in1=qh_st, op=Alu.subtract
                        )
                        nc.sync.dma_start_transpose(qlT[:, js], ql_st)

                # ---- gemm1: energy[c_tile, :] accumulated over n-chunks ----
                e_ps = ps_e.tile([128, C], f32, tag="ps_e")
                for j in range(NJ):
                    last = j == NJ - 1
                    nc.tensor.matmul(
                        e_ps,
                        qhT[:, j, :],
                        khT[:, :, j, :],
                        start=(j == 0),
                        stop=(last and not q_split and not kv_split),
                    )
                    if kv_split:
                        # same stationary (qhT) as the previous matmul
                        nc.tensor.matmul(
                            e_ps, qhT[:, j, :], klT[:, :, j, :], start=False,
                            stop=(last and not q_split),
                        )
                    if q_split:
                        nc.tensor.matmul(
                            e_ps,
                            qlT[:, j, :],
                            khT[:, :, j, :],
                            start=False,
                            stop=last,
                        )

                # ---- softmax over d (free axis): att = exp(min - E)/sum ----
                min_t = small.tile([128, 1], f32, tag="min")
                nc.vector.tensor_reduce(min_t, e_ps, axis=Axis.X, op=Alu.min)
                att16 = att_pool.tile([128, C], f16, tag="att16")
                den = small.tile([128, 1], f32, tag="den")
                nc.scalar.activation(
                    out=att16,
                    in_=e_ps,
                    func=Act.Exp,
                    bias=min_t,
                    scale=-1.0,
                    accum_out=den,
                )
                rden = small.tile([128, 1], f32, tag="rden")
                nc.vector.reciprocal(rden, den)
                nc.vector.tensor_scalar_mul(att16, att16, rden)
                attT = att_pool.tile([128, CT, 128], f16, tag="attT")
                nc.sync.dma_start_transpose(attT, att16)

                # ---- gemm2 + final softmax over W segments ----
                for nj in range(NO):
                    o_ps = ps_o.tile([128, 512], f32, tag="ps_o")
                    for dt in range(CT):
                        nc.tensor.matmul(
                            o_ps,
                            attT[:, dt, :],
                            kh_nat[dt][:, nj * 512 : (nj + 1) * 512],
                            start=(dt == 0),
                            stop=(dt == CT - 1),
                        )
                    o_sb = opool.tile([128, 512 // W, W], f32, tag="osb")
                    nc.scalar.activation(
                        out=o_sb,
                        in_=o_ps.rearrange("p (s w) -> p s w", w=W),
                        func=Act.Exp,
                    )
                    ssum = small.tile([128, 512 // W], f32, tag="ssum")
                    nc.vector.tensor_reduce(ssum, o_sb, axis=Axis.X, op=Alu.add)
                    rsum = small.tile([128, 512 // W], f32, tag="rsum")
                    nc.vector.reciprocal(rsum, ssum)
                    nc.vector.tensor_tensor(
                        out=o_sb,
                        in0=o_sb,
                        in1=rsum[:, :, None].to_broadcast(o_sb.shape),
                        op=Alu.mult,
                    )
                    nc.sync.dma_start(
                        out=out[
                            b, ct * 128 : (ct + 1) * 128, nj * 512 : (nj + 1) * 512
                        ],
                        in_=o_sb,
                    )

    nc.finalize()
    return nc


def kernel(x_training: np.ndarray, x_pre: np.ndarray) -> np.ndarray:
    from concourse.bass_utils import run_bass_kernel_spmd

    nc = build_program()

    xt = np.ascontiguousarray(
        np.asarray(x_training, dtype=np.float32).reshape(B, C, N)
    )
    xp = np.ascontiguousarray(np.asarray(x_pre, dtype=np.float32).reshape(B, C, N))

    in_maps = []
    for i in range(N_CORES):
        sl = slice(i * B_PER_CORE, (i + 1) * B_PER_CORE)
        in_maps.append({"xt": xt[sl], "xp": xp[sl]})

    res = run_bass_kernel_spmd(nc, in_maps, list(range(N_CORES)))
    outs = [np.asarray(r["out"]) for r in res.results]
    return np.concatenate(outs, axis=0).reshape(B, C, H, W).astype(np.float32)
